# revision 1
# baseline (speedup 1.0000x reference)
"""Trainium2 Bass kernel for batched attention with query-axis softmax.

Reference computation (per example b of 64):
    Q = q @ Wq.T + bq              # [S=1024, Y=128]
    K = q @ Wk.T + bk
    V = q @ Wv.T + bv
    scores = Q @ K.T / sqrt(Y)     # [Sq, Sk]
    attn   = softmax(scores, axis=-2)   # normalize over the QUERY axis
    out    = attn @ V              # [S, Y]
    result = max(out, axis=-2)     # [Y]

Key structural facts exploited here:
  * softmax normalizes over q, which is NOT the contraction axis of attn@V:
    out[q,d] = sum_k U[q,k]/c[k] * V[k,d] with U = exp(scores),
    c[k] = sum_q U[q,k].  So the normalization folds into V's rows:
    out = U @ (V / c).  No SxS division needed.
  * storing scores transposed (scoresT[k,q]) makes c a free-dim row-sum,
    which the ScalarE Exp instruction produces for free via accum_out.
  * outT[d,q] = V'.T-accumulated matmul keeps the final max a free-dim
    reduce_max -> [128,1] per example.

The kernel is ACT-bound: 64 exp instructions of [128,1024] plus their
accumulator reads pace the whole pipeline at ~1.24us/k-tile, so the span
is startup + 64 x 1.24us + tail.  This version therefore minimizes the
startup latency: the activation table loads during the input DMA (dummy
exp), the PE clock is pre-warmed with garbage matmuls so the HAM
throttle flips to 2.4 GHz during the first projection, example 0 arrives
in two half-DMAs consumed by a chunk-granular projection with per-half
drains, and examples 1-2 prefetch only after example 0's halves are in
flight (they otherwise contend for HBM with all 8 cores bursting).

All matmul operands are fp16; accumulation is fp32 in PSUM and the
softmax sums/normalization are fp32.

Sharding: data-parallel over batch, 8 examples per NeuronCore x 8 cores.
"""

import numpy as np
from contextlib import ExitStack

import concourse.bacc as bacc
import concourse.tile as tile
import concourse.mybir as mybir
import concourse.bass_utils as bass_utils

F32 = mybir.dt.float32
BF16 = mybir.dt.float16  # 16-bit matmul dtype: fp16 (11-bit significand)

NCORES = 8
B_PER_CORE = 8
S = 1024          # sequence length
X = 256           # input dim
Y = 128           # head dim
P = 128           # partitions
NH = 2            # 512-column halves of S (psum bank limit)
NKT = S // P      # 8 k-tiles


def emit(ctx, tc, out_d, ins):
    nc = tc.nc
    AF = mybir.ActivationFunctionType
    AX = mybir.AxisListType

    qt_d, w_d, b_d = ins

    wpool = ctx.enter_context(tc.tile_pool(name="w", bufs=1))
    qtp = ctx.enter_context(tc.tile_pool(name="qtp", bufs=4))
    qkp = ctx.enter_context(tc.tile_pool(name="qk", bufs=2))
    up = ctx.enter_context(tc.tile_pool(name="u", bufs=11))
    vrp = ctx.enter_context(tc.tile_pool(name="vr", bufs=4))
    vsp = ctx.enter_context(tc.tile_pool(name="vs", bufs=11))
    crp = ctx.enter_context(tc.tile_pool(name="cr", bufs=12))
    resp = ctx.enter_context(tc.tile_pool(name="res", bufs=1))
    # PSUM budget (8 banks): scores 2x2 + proj 1 + attnV-accum 2 + V 1
    pmm = ctx.enter_context(tc.tile_pool(name="pmm", bufs=2, space="PSUM"))
    pprj = ctx.enter_context(tc.tile_pool(name="pprj", bufs=1, space="PSUM"))
    pout = ctx.enter_context(tc.tile_pool(name="pout", bufs=1, space="PSUM"))
    pvp = ctx.enter_context(tc.tile_pool(name="pv", bufs=1, space="PSUM"))

    # Warm the PE clock (HAM) during the input DMA: garbage matmuls ahead
    # of the first projection.  The source memset is the first gpsimd
    # instruction so the warmup starts right after the engine preambles.
    wsrc = wpool.tile([P, 512], BF16)
    nc.gpsimd.memset(wsrc[:], 0)
    pwarm = pvp.tile([P, 512], F32, tag="pv")
    for _ in range(5):
        nc.tensor.matmul(pwarm[:], lhsT=wsrc[:, 0:P], rhs=wsrc[:],
                         start=True, stop=True)

    # Dummy activation: walrus places ACT_TABLE_LOAD here, during the DMA.
    scr2 = wpool.tile([P, 1], F32)
    nc.scalar.activation(scr2[:], wsrc[:, 0:1], AF.Exp)

    # Constants first: w on gpsimd, bqk on the scalar HWDGE queue so both
    # transfers run during the framework preamble.
    # w: [128, 3*256] bf16 -- wq | wk | wv, each [128, 2*Y] (x-chunk xb at
    #    columns xb*Y..), projection scale folded into wq.
    # b: [128, 2+128] f32 -- bq_scaled | bk | identity (for final transpose)
    w = wpool.tile([P, 7 * Y], BF16)
    nc.gpsimd.dma_start(w[:, 0:4 * Y], w_d[:, 0:4 * Y])
    nc.gpsimd.dma_start(w[:, 4 * Y:7 * Y], w_d[:, 4 * Y:7 * Y])
    bqk = wpool.tile([P, 2 + P], F32)
    wq = w[:, 0 * Y: 2 * Y]
    wk = w[:, 2 * Y: 4 * Y]
    wv = w[:, 4 * Y: 6 * Y]



    def load_qt(b, eng):
        # qT[b] : [256, 1024] -> sbuf [128, 2*1024], x-chunk xb at cols xb*S..
        qt = qtp.tile([P, 2 * S], BF16, tag="qt")
        qv = qt_d[b].rearrange("(xb p) s -> p xb s", p=P)
        eng.dma_start(qt[:].rearrange("p (xb s) -> p xb s", xb=2), qv)
        return qt

    def proj_half(qt, dst, w_sb, bcol, nh):
        # One 512-column half of a Q/K projection: ZT[y, s_half] = W.T @ qT
        pm = pprj.tile([P, 512], F32, tag="pj")
        for xb in range(2):
            nc.tensor.matmul(
                pm[:],
                lhsT=w_sb[:, xb * Y:(xb + 1) * Y],
                rhs=qt[:, xb * S + nh * 512: xb * S + nh * 512 + 512],
                start=(xb == 0),
                stop=(xb == 1),
            )
        # psum -> sbuf with per-partition bias
        nc.vector.tensor_scalar_add(
            dst[:, nh * 512:(nh + 1) * 512], pm[:], bqk[:, bcol:bcol + 1]
        )

    vstiles = {}   # (b, kt) -> [128, 128] V' tile
    utiles = {}    # (b, kt) -> exp tile

    def front(qt, QT, KT, b, kt):
        """scores -> exp(+colsum) -> V -> V/c for one k-tile."""
        ps = pmm.tile([P, S], F32, tag="mm")
        u = up.tile([P, S], BF16, tag="u")
        c = crp.tile([P, 1], F32, tag="c")
        if b == 0 and kt == 0:
            # Very first k-tile: run scores+exp per 512-half so the exp
            # stream starts after only half the projection has drained
            # (the h1 input DMA + projection finish in its shadow).
            # c = c_h0 + c_h1 stitched with one DVE add.
            ch = [crp.tile([P, 1], F32, tag="c", name="ch") for _ in range(2)]
            with tc.high_priority(offset=40):
                for nh in range(NH):
                    nc.tensor.matmul(
                        ps[:, nh * 512:(nh + 1) * 512],
                        lhsT=KT[:, kt * P:(kt + 1) * P],
                        rhs=QT[:, nh * 512: nh * 512 + 512],
                        start=True,
                        stop=True,
                    )
                    nc.scalar.activation(
                        u[:, nh * 512:(nh + 1) * 512],
                        ps[:, nh * 512:(nh + 1) * 512],
                        AF.Exp, accum_out=ch[nh][:])
            nc.vector.tensor_add(c[:], ch[0][:], ch[1][:])
        else:
            with tc.high_priority(offset=40):
                for nh in range(NH):
                    nc.tensor.matmul(
                        ps[:, nh * 512:(nh + 1) * 512],
                        lhsT=KT[:, kt * P:(kt + 1) * P],
                        rhs=QT[:, nh * 512: nh * 512 + 512],
                        start=True,
                        stop=True,
                    )
                # U = exp(scoresT), c = sum_q U (free accumulation on ACT)
                nc.scalar.activation(u[:], ps[:], AF.Exp, accum_out=c[:])
        utiles[(b, kt)] = u

        # V k-tile directly in [k, d] layout: V[s_tile,:] =
        #   qT_chunk.T @ WvT (+ bias via partition-broadcast on drain)
        pv = pvp.tile([P, P], F32, tag="pv")
        for xb in range(2):
            nc.tensor.matmul(
                pv[:],
                lhsT=qt[:, xb * S + kt * P: xb * S + (kt + 1) * P],
                rhs=wv[:, xb * Y:(xb + 1) * Y],
                start=(xb == 0),
                stop=(xb == 1),
            )
        vraw = vrp.tile([P, P], BF16, tag="vr")
        nc.vector.tensor_add(vraw[:], pv[:], w[:, 6 * Y:7 * Y])

        # V'[k, :] = V[k, :] / c[k]
        r = crp.tile([P, 1], F32, tag="r")
        nc.vector.reciprocal(r[:], c[:])
        vs = vsp.tile([P, P], BF16, tag="vs")
        nc.vector.tensor_scalar_mul(vs[:], vraw[:], r[:])
        vstiles[(b, kt)] = vs

    # Software-pipelined emission over a flat (b, kt) step stream.  The
    # attnV accumulation runs LAG steps behind the scores->exp front so the
    # in-order PE always has the next exp's scores queued ahead of
    # slack-tolerant work (keeps ACT, the bottleneck engine, saturated), and
    # example b+1's DMA + projections are emitted inside example b's k-loop.
    LAG = 4
    steps = [(b, kt) for b in range(B_PER_CORE) for kt in range(NKT)]
    state = {}       # b -> (qt, QT, KT)
    fifo = {}        # step index -> (b, kt)
    po = None

    # Example 0 input in two 512-column halves: h0 on the sync queue,
    # h1 on the scalar queue, so the first projection half starts as soon
    # as possible (each DMA trigger instruction alone costs ~750ns).
    qt0 = qtp.tile([P, 2 * S], BF16, tag="qt")
    qv0 = qt_d[0].rearrange("(xb p) s -> p xb s", p=P)
    qt0v = qt0[:].rearrange("p (xb s) -> p xb s", xb=2)
    nc.sync.dma_start(qt0v[:, :, 0:512], qv0[:, :, 0:512])
    nc.sync.dma_start(qt0v[:, :, 512:1024], qv0[:, :, 512:1024])
    nc.sync.dma_start(bqk[:], b_d[:])

    # Example 0 projection, consuming chunks in arrival order with
    # per-half drains.
    QT0 = qkp.tile([P, S], BF16, tag="QT")
    KT0 = qkp.tile([P, S], BF16, tag="KT")
    pmQ = pmm.tile([P, S], F32, tag="mm")
    pmK = pmm.tile([P, S], F32, tag="mm")
    with tc.high_priority():
        for h in range(2):
            for xb in range(2):
                for pm, w_sb in ((pmQ, wq), (pmK, wk)):
                    nc.tensor.matmul(
                        pm[:, h * 512: h * 512 + 512],
                        lhsT=w_sb[:, xb * Y:(xb + 1) * Y],
                        rhs=qt0[:, xb * S + h * 512: xb * S + h * 512 + 512],
                        start=(xb == 0),
                        stop=(xb == 1),
                    )
            nc.vector.tensor_scalar_add(
                QT0[:, h * 512: h * 512 + 512],
                pmQ[:, h * 512: h * 512 + 512], bqk[:, 0:1])
            nc.vector.tensor_scalar_add(
                KT0[:, h * 512: h * 512 + 512],
                pmK[:, h * 512: h * 512 + 512], bqk[:, 1:2])
    state[0] = (qt0, QT0, KT0)

    res_all = resp.tile([P, B_PER_CORE], F32, tag="res")

    def drain(i):
        nonlocal po
        b, kt = fifo.pop(i)
        u = utiles.pop((b, kt))
        vs = vstiles.pop((b, kt))
        if kt == 0:
            po = pout.tile([P, S], F32, tag="out")
        # outT[d, q] += V'.T @ U   (contract k)
        for nh in range(NH):
            nc.tensor.matmul(
                po[:, nh * 512:(nh + 1) * 512],
                lhsT=vs[:],
                rhs=u[:, nh * 512: nh * 512 + 512],
                start=(kt == 0),
                stop=(kt == NKT - 1),
            )
        if kt == NKT - 1:
            # high priority: the next example's first attnV drain waits on
            # this to free the out-psum bank; run it ahead of the V chain.
            with tc.high_priority(offset=40):
                nc.vector.reduce_max(res_all[:, b:b + 1], po[:], axis=AX.X)

    qtiles = {0: qt0}

    for i, (b, kt) in enumerate(steps):
        qt, QT, KT = state[b]
        if b == 0 and kt in (0, 1):
            # deferred prefetch: don't let examples 1-2 contend with
            # example 0's own chunks for HBM bandwidth at startup
            qtiles[kt + 1] = load_qt(kt + 1, nc.sync)
        if kt == 0 and b + 1 < B_PER_CORE:
            state[b + 1] = (qtiles[b + 1],)
        if kt == 1 and b + 3 < B_PER_CORE:
            qtiles[b + 3] = load_qt(b + 3, nc.sync)
        if kt == 2 and b + 1 < B_PER_CORE:
            # allocate next example's projection outputs; halves fill in
            # one per step over kt=2..5
            QT_n = qkp.tile([P, S], BF16, tag="QT")
            KT_n = qkp.tile([P, S], BF16, tag="KT")
            state[b + 1] = (state[b + 1][0], QT_n, KT_n)
        if 2 <= kt <= 5 and b + 1 < B_PER_CORE:
            qt_n, QT_n, KT_n = state[b + 1]
            w_sb, bcol, dst = ((wq, 0, QT_n), (wk, 1, KT_n))[(kt - 2) // 2]
            proj_half(qt_n, dst, w_sb, bcol, (kt - 2) % 2)
        front(qt, QT, KT, b, kt)
        fifo[i] = (b, kt)
        target = i - LAG
        if b == B_PER_CORE - 1 and kt >= 4:
            target = i - LAG + (kt - 3)  # taper: catch up 2/step at the end
        while fifo and min(fifo) <= target:
            drain(min(fifo))
    for i in sorted(fifo):
        drain(i)

    # Ship the collected [128(d), 8(b)] results directly: 128 rows of 32B
    # to a contiguous [128, 8] DRAM tensor (host transposes for free).
    # Skips the PE transpose + DVE copy on the critical tail.
    nc.sync.dma_start(out_d[:], res_all[:])


def build_program():
    nc = bacc.Bacc(
        "TRN2",
        target_bir_lowering=False,
        debug=False,
        enable_asserts=False,
    )
    qt = nc.dram_tensor("qt", [B_PER_CORE, X, S], BF16, kind="ExternalInput").ap()
    w = nc.dram_tensor("w", [P, 7 * Y], BF16, kind="ExternalInput").ap()
    b = nc.dram_tensor("b", [P, 2 + P], F32, kind="ExternalInput").ap()
    out = nc.dram_tensor("out", [P, B_PER_CORE], F32, kind="ExternalOutput").ap()

    ins = (qt, w, b)
    with tile.TileContext(nc) as tc:
        with ExitStack() as ctx:
            emit(ctx, tc, out, ins)
    nc.compile()
    return nc


_NC_CACHE = None


def _get_program():
    global _NC_CACHE
    if _NC_CACHE is None:
        _NC_CACHE = build_program()
    return _NC_CACHE


def prep_inputs(q, Wq, bq, Wk, bk, Wv, bv):
    """Host-side marshalling: transpose q, pack weights, fold softmax scale."""
    q = np.asarray(q, dtype=np.float32)
    scale = np.float32(1.0 / np.sqrt(Y))
    f16 = np.float16

    qT = np.ascontiguousarray(q.transpose(0, 2, 1)).astype(f16)  # [B, X, S]

    def pack(w):  # [Y, X] torch layout -> [128, 2*Y]: chunk xb at cols xb*Y..
        wt = np.asarray(w, dtype=np.float32).T  # [X, Y]
        return np.concatenate([wt[0:P], wt[P:2 * P]], axis=1)

    w_all = np.concatenate(
        [pack(Wq) * scale, pack(Wk), pack(Wv),
         np.tile(np.asarray(bv, np.float32).reshape(1, Y), (P, 1))], axis=1
    ).astype(f16)
    b_all = np.concatenate(
        [np.stack([np.asarray(bq, np.float32) * scale,
                   np.asarray(bk, np.float32)], axis=1),
         np.eye(P, dtype=np.float32)], axis=1
    ).astype(np.float32)
    feeds = {
        "w": np.ascontiguousarray(w_all),
        "b": np.ascontiguousarray(b_all),
    }
    return qT, feeds


def kernel(q, Wq, bq, Wk, bk, Wv, bv, _trace=False):
    qT, feeds = prep_inputs(q, Wq, bq, Wk, bk, Wv, bv)
    nc = _get_program()
    in_maps = [
        {"qt": qT[c * B_PER_CORE:(c + 1) * B_PER_CORE], **feeds}
        for c in range(NCORES)
    ]
    kw = {}
    if _trace:
        kw = dict(trace=True)
    res = bass_utils.run_bass_kernel_spmd(
        nc, in_maps, core_ids=list(range(NCORES)), **kw
    )
    out = np.concatenate([np.ascontiguousarray(r["out"].T) for r in res.results], axis=0)
    if _trace:
        return out, res
    return out

